# revision 10
# baseline (speedup 1.0000x reference)
"""APPNP (10-hop propagation) on 8 TRN2 NeuronCores.

Strategy: shard destination nodes across the 8 cores (6250 each, padded to
6272 = 49*128). Per hop and per core: dma_gather the messages u[src] for the
core's in-edges (tokens pre-sorted by dst on the host), segment-sum them on
the TensorEngine via one-hot matrices generated ON-CHIP (DVE is_equal of a
preloaded per-token dst vector against an iota row), apply the teleport
update, and AllGather the new u shards into replicated HBM tables for the
next hop's gather.

Each shard is split at row 3200 into an A half (rows 0:3200, blocks 0-24)
and a B half (rows 3200:6272, blocks 25-48). The halves are AllGathered
separately: AG-A fires mid-hop (right after blocks 0-24 are computed) and
overlaps the second half of the hop; AG-B fires at hop end and hides under
the next hop's A-stream gathers (which only depend on AG-A). Both gathered
tables (25600 and 24576 rows) are addressable with int16 indices, so no
lo/hi index split is needed. Tokens are packed densely; chunks that straddle
a dst-block boundary issue one extra matmul whose Q columns auto-mask via
the range compare.
"""
import os
import sys

sys.path.insert(0, '/opt/trn_rl_repo')

import numpy as np

N = 50000
D = 64
E = 800000
K = 10
ALPHA = 0.1
C = 8                 # cores
NS = 6250             # real dst nodes per core
NSP = 6272            # padded (49 * 128)
NB = 49               # dst blocks per core
HALFA = 3200          # rows per core in the A half (blocks 0-24)
HALFB = NSP - HALFA   # 3072 rows, blocks 25-48
NBA = HALFA // 128    # 25
NPA = C * HALFA       # 25600 rows in table A
NPB = C * HALFB       # 24576 rows in table B
SGB = 5               # dst blocks per supergroup
SGS = [(b, min(b + SGB, NB)) for b in range(0, NB, SGB)]
NSG = len(SGS)


def _host_prep(x, edge_index):
    import ml_dtypes

    src = np.asarray(edge_index[0], dtype=np.int64)
    dst = np.asarray(edge_index[1], dtype=np.int64)
    x = np.asarray(x, dtype=np.float32)

    deg = np.bincount(dst, minlength=N).astype(np.float64) + 1.0
    dinv = 1.0 / np.sqrt(deg)

    src_core = src // NS
    src_row = src % NS
    # half-table row index of each edge's source
    is_a = src_row < HALFA
    rowA = src_core * HALFA + src_row
    rowB = src_core * HALFB + (src_row - HALFA)

    core = dst // NS
    dst_local = dst - core * NS

    # ---- per (core, supergroup, half) dst-sorted token streams ----
    toks = {}   # (c, s, h) -> (table_row int64, dstloc int64)
    for c in range(C):
        mc = core == c
        ra_c = rowA[mc]
        rb_c = rowB[mc]
        dl_c = dst_local[mc]
        a_c = is_a[mc]
        blk = dl_c // 128
        for s, (b0, b1) in enumerate(SGS):
            ms = (blk >= b0) & (blk < b1)
            for h in range(2):
                mh = ms & (a_c if h == 0 else ~a_c)
                rr = (ra_c if h == 0 else rb_c)[mh]
                dl = dl_c[mh]
                order = np.argsort(dl, kind='stable')
                toks[(c, s, h)] = (rr[order], dl[order])

    # static (shared across cores) stream sizes, rounded to 128
    T = np.zeros((NSG, 2), dtype=np.int64)
    for s in range(NSG):
        for h in range(2):
            mx = max(len(toks[(c, s, h)][0]) for c in range(C))
            T[s, h] = ((mx + 127) // 128) * 128
    NCH = T // 128

    # ---- static MM instance table ----
    mm_off = np.zeros((NSG, 2), dtype=np.int64)
    nmm = np.zeros((NSG, 2), dtype=np.int64)
    chains = []        # per sg: dict b -> list[(h, ch, local_col)]
    inst_list = []     # per (s, h): list of (ch, b) in col order
    off = 0
    for s, (b0, b1) in enumerate(SGS):
        ch_map = {b: [] for b in range(b0, b1)}
        per_sh = []
        for h in range(2):
            mm_off[s, h] = off
            insts = []
            for k in range(NCH[s, h]):
                blocks = set()
                for c in range(C):
                    dl = toks[(c, s, h)][1]
                    seg = dl[k * 128:(k + 1) * 128]
                    if len(seg):
                        blocks.update((seg // 128).tolist())
                for b in sorted(blocks):
                    insts.append((k, b))
            for j, (k, b) in enumerate(insts):
                ch_map[b].append((h, k, j))
            nmm[s, h] = len(insts)
            off += len(insts)
            per_sh.append(insts)
        chains.append(ch_map)
        inst_list.append(per_sh)
    NMM = int(off)

    # ---- per-core qv + idx arrays ----
    qv = np.full((C, 128, NMM), -1.0, dtype=np.float32)
    TA = int(T[:, 0].sum())
    TB = int(T[:, 1].sum())
    idx_a = np.zeros((C, TA), dtype=np.int16)
    idx_b = np.zeros((C, TB), dtype=np.int16)
    t_off = np.zeros((NSG, 2), dtype=np.int64)
    oa = 0
    ob = 0
    for s in range(NSG):
        t_off[s, 0] = oa
        t_off[s, 1] = ob
        oa += int(T[s, 0])
        ob += int(T[s, 1])

    for c in range(C):
        for s in range(NSG):
            for h in range(2):
                rr, dl = toks[(c, s, h)]
                n = len(rr)
                o = int(t_off[s, h])
                if h == 0:
                    idx_a[c, o:o + n] = rr.astype(np.int16)
                else:
                    idx_b[c, o:o + n] = rr.astype(np.int16)
                # pads stay 0 (fetch row 0; Q col is zero)
                dpad = np.full(int(T[s, h]), -1.0, dtype=np.float32)
                dpad[:n] = dl.astype(np.float32)
                for j, (k, b) in enumerate(inst_list[s][h]):
                    col = int(mm_off[s, h]) + j
                    seg = dpad[k * 128:(k + 1) * 128]
                    qv[c, :len(seg), col] = seg - 128.0 * b

    qv16 = qv.astype(ml_dtypes.bfloat16)

    def wrap16(a):
        n = a.shape[-1]
        w = a.reshape(C, n // 16, 16).transpose(0, 2, 1)
        return np.tile(w, (1, 8, 1)).copy()

    idx_a_w = wrap16(idx_a)
    idx_b_w = wrap16(idx_b)

    iota = np.broadcast_to(np.arange(128, dtype=np.float32), (128, 128))
    iota16 = np.ascontiguousarray(iota).astype(ml_dtypes.bfloat16)

    # per-core node-level tensors, padded to NSP rows
    u0 = (dinv[:, None] * x).astype(np.float32)
    y0 = (ALPHA * dinv[:, None] * x).astype(np.float32)
    w_full = (0.9 * dinv * dinv).astype(np.float32)
    sq_full = np.sqrt(deg).astype(np.float32)

    def shard_pad2(a2d):
        out = np.zeros((C, NSP, D), dtype=np.float32)
        for c in range(C):
            out[c, :NS] = a2d[c * NS:(c + 1) * NS]
        return out

    def shard_pad1(a1d):
        out = np.zeros((C, NSP), dtype=np.float32)
        for c in range(C):
            out[c, :NS] = a1d[c * NS:(c + 1) * NS]
        return out

    u0_s = shard_pad2(u0)
    y0_s = shard_pad2(y0)
    w_s = shard_pad1(w_full)
    sq_s = shard_pad1(sq_full)

    meta = dict(T=T, NCH=NCH, mm_off=mm_off, nmm=nmm, chains=chains,
                t_off=t_off, NMM=NMM, TA=TA, TB=TB)
    return meta, idx_a_w, idx_b_w, qv16, iota16, u0_s, y0_s, w_s, sq_s


def _build_nc(meta):
    import concourse.bacc as bacc
    import concourse.mybir as mybir
    import concourse.tile as tile

    T, NCH = meta['T'], meta['NCH']
    mm_off, nmm = meta['mm_off'], meta['nmm']
    chains = meta['chains']
    t_off = meta['t_off']
    NMM, TA, TB = meta['NMM'], meta['TA'], meta['TB']

    nc = bacc.Bacc(None, target_bir_lowering=False, num_devices=C,
                   num_swdge_queues=4)
    dt = mybir.dt.float32
    bf = mybir.dt.bfloat16

    u0_d = nc.dram_tensor("u0", [NSP, D], dt, kind="ExternalInput")
    y0_d = nc.dram_tensor("y0", [NSP, D], dt, kind="ExternalInput")
    w_d = nc.dram_tensor("w", [NSP], dt, kind="ExternalInput")
    sq_d = nc.dram_tensor("sq", [NSP], dt, kind="ExternalInput")
    ia_d = nc.dram_tensor("idx_a", [128, TA // 16], mybir.dt.int16,
                          kind="ExternalInput")
    ib_d = nc.dram_tensor("idx_b", [128, TB // 16], mybir.dt.int16,
                          kind="ExternalInput")
    qv_d = nc.dram_tensor("qv", [128, NMM], bf, kind="ExternalInput")
    iota_d = nc.dram_tensor("iota", [128, 128], bf, kind="ExternalInput")
    out_d = nc.dram_tensor("out", [NSP, D], dt, kind="ExternalOutput")

    bounceA = nc.dram_tensor("bounceA", [HALFA, D], dt)
    bounceB = nc.dram_tensor("bounceB", [HALFB, D], dt)
    urepsA = [nc.dram_tensor(f"urepA{i}", [NPA, D], dt, addr_space="Shared")
              for i in range(2)]
    urepsB = [nc.dram_tensor(f"urepB{i}", [NPB, D], dt, addr_space="Shared")
              for i in range(2)]

    with tile.TileContext(nc) as tc:
        with (
            tc.tile_pool(name="res", bufs=1) as res,
            tc.tile_pool(name="mbuf", bufs=2) as mpool,
            tc.tile_pool(name="m16buf", bufs=2) as m16pool,
            tc.tile_pool(name="qbuf", bufs=2) as qpool,
            tc.tile_pool(name="psum", bufs=8, space="PSUM") as ppool,
        ):
            uA = res.tile([128, NB, D], dt, tag="uA")
            uB = res.tile([128, NB, D], dt, tag="uB")
            y0t = res.tile([128, NB, D], dt, tag="y0t")
            wt = res.tile([128, NB], dt, tag="wt")
            sqt = res.tile([128, NB], dt, tag="sqt")
            iota_t = res.tile([128, 128], bf, tag="iota")
            qv_t = res.tile([128, NMM], bf, tag="qv")
            ia_t = res.tile([128, TA // 16], mybir.dt.int16, tag="ia")
            ib_t = res.tile([128, TB // 16], mybir.dt.int16, tag="ib")

            def node_ap(dram):
                return dram[:].rearrange("(b p) f -> p b f", p=128)

            def node_ap1(dram):
                return dram[:].rearrange("(b p) -> p b", p=128)

            nc.sync.dma_start(uA[:], node_ap(u0_d))
            nc.sync.dma_start(y0t[:], node_ap(y0_d))
            nc.sync.dma_start(wt[:], node_ap1(w_d))
            nc.sync.dma_start(sqt[:], node_ap1(sq_d))
            nc.sync.dma_start(iota_t[:], iota_d[:])
            nc.sync.dma_start(qv_t[:], qv_d[:])
            nc.sync.dma_start(ia_t[:], ia_d[:])
            nc.sync.dma_start(ib_t[:], ib_d[:])

            # initial AllGather of u0 halves
            nc.sync.dma_start(bounceA[:], u0_d[0:HALFA, :])
            nc.sync.dma_start(bounceB[:], u0_d[HALFA:NSP, :])
            nc.gpsimd.collective_compute(
                "AllGather", mybir.AluOpType.bypass,
                replica_groups=[list(range(C))],
                ins=[bounceA[:]], outs=[urepsA[0][:]],
            )
            nc.gpsimd.collective_compute(
                "AllGather", mybir.AluOpType.bypass,
                replica_groups=[list(range(C))],
                ins=[bounceB[:]], outs=[urepsB[0][:]],
            )

            u_cur, u_nxt = uA, uB
            qn = [0]
            for hop in range(K):
                viewA = urepsA[hop % 2][:]
                viewB = urepsB[hop % 2][:]
                for s, (b0, b1) in enumerate(SGS):
                    ta = int(T[s, 0])
                    tb = int(T[s, 1])
                    nca = int(NCH[s, 0])
                    ncb = int(NCH[s, 1])
                    sa = int(t_off[s, 0])
                    sb = int(t_off[s, 1])

                    ma = mpool.tile([128, nca, D], dt, tag="ma")
                    mb = mpool.tile([128, ncb, D], dt, tag="mb")
                    # split each stream's gather into ring-sized pieces so
                    # SWDGE descriptor generation never blocks on ring drain
                    GS = 768
                    for g0 in range(0, ta, GS):
                        g1 = min(g0 + GS, ta)
                        nc.gpsimd.dma_gather(
                            ma[:, g0 // 128:g1 // 128, :], viewA,
                            ia_t[:, (sa + g0) // 16:(sa + g1) // 16],
                            g1 - g0, g1 - g0, D, single_packet=False,
                            queue_num=qn[0] % 4)
                        qn[0] += 1
                    for g0 in range(0, tb, GS):
                        g1 = min(g0 + GS, tb)
                        nc.gpsimd.dma_gather(
                            mb[:, g0 // 128:g1 // 128, :], viewB,
                            ib_t[:, (sb + g0) // 16:(sb + g1) // 16],
                            g1 - g0, g1 - g0, D, single_packet=False,
                            queue_num=qn[0] % 4)
                        qn[0] += 1

                    ma16 = m16pool.tile([128, nca, D], bf, tag="ma16")
                    mb16 = m16pool.tile([128, ncb, D], bf, tag="mb16")
                    nc.scalar.copy(out=ma16[:], in_=ma[:])
                    nc.scalar.copy(out=mb16[:], in_=mb[:])

                    # on-chip one-hot generation for this supergroup
                    na = int(nmm[s, 0])
                    nb_ = int(nmm[s, 1])
                    oa = int(mm_off[s, 0])
                    ob = int(mm_off[s, 1])
                    qa = qpool.tile([128, na, 128], bf, tag="qa")
                    qb = qpool.tile([128, nb_, 128], bf, tag="qb")
                    nc.vector.tensor_tensor(
                        out=qa[:],
                        in0=qv_t[:, oa:oa + na].unsqueeze(2)
                            .broadcast_to([128, na, 128]),
                        in1=iota_t[:].unsqueeze(1)
                            .broadcast_to([128, na, 128]),
                        op=mybir.AluOpType.is_equal)
                    nc.vector.tensor_tensor(
                        out=qb[:],
                        in0=qv_t[:, ob:ob + nb_].unsqueeze(2)
                            .broadcast_to([128, nb_, 128]),
                        in1=iota_t[:].unsqueeze(1)
                            .broadcast_to([128, nb_, 128]),
                        op=mybir.AluOpType.is_equal)

                    for b in range(b0, b1):
                        chain = chains[s][b]
                        ps = ppool.tile([128, D], dt, tag="ps")
                        tot = len(chain)
                        for j, (h, k, col) in enumerate(chain):
                            qt = qa if h == 0 else qb
                            mt = ma16 if h == 0 else mb16
                            nc.tensor.matmul(
                                ps[:], qt[:, col, :], mt[:, k, :],
                                start=(j == 0), stop=(j == tot - 1))
                        # u_new = w * (agg + u) + y0
                        nc.vector.tensor_tensor(
                            out=u_nxt[:, b, :], in0=ps[:], in1=u_cur[:, b, :],
                            op=mybir.AluOpType.add)
                        # (kept as two tensor_tensor ops: the tensor_scalar
                        # family can enter 2-port DVE perf mode, which locks
                        # GpSimd out of the shared SBUF port mid-gather-gen)
                        nc.vector.tensor_tensor(
                            out=u_nxt[:, b, :], in0=u_nxt[:, b, :],
                            in1=wt[:, b:b + 1].broadcast_to([128, D]),
                            op=mybir.AluOpType.mult)
                        nc.vector.tensor_tensor(
                            out=u_nxt[:, b, :], in0=u_nxt[:, b, :],
                            in1=y0t[:, b, :],
                            op=mybir.AluOpType.add)

                    if hop < K - 1:
                        if b1 <= NBA:
                            nc.sync.dma_start(
                                node_ap(bounceA)[:, b0:b1, :],
                                u_nxt[:, b0:b1, :])
                        else:
                            nc.sync.dma_start(
                                node_ap(bounceB)[:, b0 - NBA:b1 - NBA, :],
                                u_nxt[:, b0:b1, :])
                        if b1 == NBA:
                            # blocks 0-24 done: fire AG-A mid-hop
                            nc.gpsimd.collective_compute(
                                "AllGather", mybir.AluOpType.bypass,
                                replica_groups=[list(range(C))],
                                ins=[bounceA[:]],
                                outs=[urepsA[(hop + 1) % 2][:]],
                            )

                if hop < K - 1:
                    nc.gpsimd.collective_compute(
                        "AllGather", mybir.AluOpType.bypass,
                        replica_groups=[list(range(C))],
                        ins=[bounceB[:]], outs=[urepsB[(hop + 1) % 2][:]],
                    )
                u_cur, u_nxt = u_nxt, u_cur

            # epilogue: out = relu(u * sqrt(deg))
            ot = u_nxt  # reuse the dead double buffer
            nc.vector.tensor_tensor(
                out=ot[:], in0=u_cur[:],
                in1=sqt[:].unsqueeze(2).broadcast_to([128, NB, D]),
                op=mybir.AluOpType.mult)
            nc.vector.tensor_scalar_max(out=ot[:], in0=ot[:], scalar1=0.0)
            nc.sync.dma_start(node_ap(out_d), ot[:])

    nc.compile()
    return nc


def kernel(x, edge_index):
    (meta, idx_a_w, idx_b_w, qv16, iota16,
     u0_s, y0_s, w_s, sq_s) = _host_prep(x, edge_index)
    nc = _build_nc(meta)

    from concourse.bass_utils import run_bass_kernel_spmd

    in_maps = []
    for c in range(C):
        in_maps.append({
            "u0": u0_s[c], "y0": y0_s[c], "w": w_s[c], "sq": sq_s[c],
            "idx_a": idx_a_w[c], "idx_b": idx_b_w[c],
            "qv": qv16[c], "iota": iota16,
        })

    ntff_dir = os.environ.get("APPNP_NTFF_DIR")
    if ntff_dir:
        from trn_agent_boot.trn_boot import _ntff_profile_via_ctypes
        hook = _ntff_profile_via_ctypes('/opt/axon/libaxon_pjrt.so')
        os.makedirs(ntff_dir, exist_ok=True)
        with hook(ntff_dir, None):
            res = run_bass_kernel_spmd(nc, in_maps, core_ids=list(range(C)))
    else:
        res = run_bass_kernel_spmd(nc, in_maps, core_ids=list(range(C)))

    out = np.empty((N, D), dtype=np.float32)
    for c in range(C):
        out[c * NS:(c + 1) * NS] = res.results[c]["out"][:NS]
    return out


# revision 12
# speedup vs baseline: 1.0430x; 1.0430x over previous
"""APPNP (10-hop propagation) on 8 TRN2 NeuronCores.

Strategy: shard destination nodes across the 8 cores (6250 each, padded to
6272 = 49*128). Per hop and per core: dma_gather the messages u[src] for the
core's in-edges (tokens pre-sorted by dst on the host), segment-sum them on
the TensorEngine via one-hot matrices generated ON-CHIP (DVE is_equal of a
preloaded per-token dst vector against an iota row), apply the teleport
update, and AllGather the new u shards into replicated HBM tables for the
next hop's gather.

Each shard is split at row 3200 into an A half (rows 0:3200, blocks 0-24)
and a B half (rows 3200:6272, blocks 25-48). The halves are AllGathered
separately: AG-A fires mid-hop (right after blocks 0-24 are computed) and
overlaps the second half of the hop; AG-B fires at hop end and hides under
the next hop's A-stream gathers (which only depend on AG-A). Both gathered
tables (25600 and 24576 rows) are addressable with int16 indices, so no
lo/hi index split is needed. Tokens are packed densely; chunks that straddle
a dst-block boundary issue one extra matmul whose Q columns auto-mask via
the range compare.
"""
import os
import sys

sys.path.insert(0, '/opt/trn_rl_repo')

import numpy as np

N = 50000
D = 64
E = 800000
K = 10
ALPHA = 0.1
C = 8                 # cores
NS = 6250             # real dst nodes per core
NSP = 6272            # padded (49 * 128)
NB = 49               # dst blocks per core
HALFA = 3200          # rows per core in the A half (blocks 0-24)
HALFB = NSP - HALFA   # 3072 rows, blocks 25-48
NBA = HALFA // 128    # 25
NPA = C * HALFA       # 25600 rows in table A
NPB = C * HALFB       # 24576 rows in table B
SGB = 5               # dst blocks per supergroup
SGS = [(b, min(b + SGB, NB)) for b in range(0, NB, SGB)]
NSG = len(SGS)


def _host_prep(x, edge_index):
    import ml_dtypes

    src = np.asarray(edge_index[0], dtype=np.int64)
    dst = np.asarray(edge_index[1], dtype=np.int64)
    x = np.asarray(x, dtype=np.float32)

    deg = np.bincount(dst, minlength=N).astype(np.float64) + 1.0
    dinv = 1.0 / np.sqrt(deg)

    src_core = src // NS
    src_row = src % NS
    # half-table row index of each edge's source
    is_a = src_row < HALFA
    rowA = src_core * HALFA + src_row
    rowB = src_core * HALFB + (src_row - HALFA)

    core = dst // NS
    dst_local = dst - core * NS

    # ---- per (core, supergroup, half) dst-sorted token streams ----
    toks = {}   # (c, s, h) -> (table_row int64, dstloc int64)
    for c in range(C):
        mc = core == c
        ra_c = rowA[mc]
        rb_c = rowB[mc]
        dl_c = dst_local[mc]
        a_c = is_a[mc]
        blk = dl_c // 128
        for s, (b0, b1) in enumerate(SGS):
            ms = (blk >= b0) & (blk < b1)
            for h in range(2):
                mh = ms & (a_c if h == 0 else ~a_c)
                rr = (ra_c if h == 0 else rb_c)[mh]
                dl = dl_c[mh]
                order = np.argsort(dl, kind='stable')
                toks[(c, s, h)] = (rr[order], dl[order])

    # static (shared across cores) stream sizes, rounded to 128
    T = np.zeros((NSG, 2), dtype=np.int64)
    for s in range(NSG):
        for h in range(2):
            mx = max(len(toks[(c, s, h)][0]) for c in range(C))
            T[s, h] = ((mx + 127) // 128) * 128
    NCH = T // 128

    # ---- static MM instance table ----
    mm_off = np.zeros((NSG, 2), dtype=np.int64)
    nmm = np.zeros((NSG, 2), dtype=np.int64)
    chains = []        # per sg: dict b -> list[(h, ch, local_col)]
    inst_list = []     # per (s, h): list of (ch, b) in col order
    off = 0
    for s, (b0, b1) in enumerate(SGS):
        ch_map = {b: [] for b in range(b0, b1)}
        per_sh = []
        for h in range(2):
            mm_off[s, h] = off
            insts = []
            for k in range(NCH[s, h]):
                blocks = set()
                for c in range(C):
                    dl = toks[(c, s, h)][1]
                    seg = dl[k * 128:(k + 1) * 128]
                    if len(seg):
                        blocks.update((seg // 128).tolist())
                for b in sorted(blocks):
                    insts.append((k, b))
            for j, (k, b) in enumerate(insts):
                ch_map[b].append((h, k, j))
            nmm[s, h] = len(insts)
            off += len(insts)
            per_sh.append(insts)
        chains.append(ch_map)
        inst_list.append(per_sh)
    NMM = int(off)

    # ---- per-core qv + idx arrays ----
    qv = np.full((C, 128, NMM), -1.0, dtype=np.float32)
    TA = int(T[:, 0].sum())
    TB = int(T[:, 1].sum())
    idx_a = np.zeros((C, TA), dtype=np.int16)
    idx_b = np.zeros((C, TB), dtype=np.int16)
    t_off = np.zeros((NSG, 2), dtype=np.int64)
    oa = 0
    ob = 0
    for s in range(NSG):
        t_off[s, 0] = oa
        t_off[s, 1] = ob
        oa += int(T[s, 0])
        ob += int(T[s, 1])

    for c in range(C):
        for s in range(NSG):
            for h in range(2):
                rr, dl = toks[(c, s, h)]
                n = len(rr)
                o = int(t_off[s, h])
                if h == 0:
                    idx_a[c, o:o + n] = rr.astype(np.int16)
                else:
                    idx_b[c, o:o + n] = rr.astype(np.int16)
                # pads stay 0 (fetch row 0; Q col is zero)
                dpad = np.full(int(T[s, h]), -1.0, dtype=np.float32)
                dpad[:n] = dl.astype(np.float32)
                for j, (k, b) in enumerate(inst_list[s][h]):
                    col = int(mm_off[s, h]) + j
                    seg = dpad[k * 128:(k + 1) * 128]
                    qv[c, :len(seg), col] = seg - 128.0 * b

    qv16 = qv.astype(ml_dtypes.bfloat16)

    def wrap16(a):
        n = a.shape[-1]
        w = a.reshape(C, n // 16, 16).transpose(0, 2, 1)
        return np.tile(w, (1, 8, 1)).copy()

    idx_a_w = wrap16(idx_a)
    idx_b_w = wrap16(idx_b)

    iota = np.broadcast_to(np.arange(128, dtype=np.float32), (128, 128))
    iota16 = np.ascontiguousarray(iota).astype(ml_dtypes.bfloat16)

    # per-core node-level tensors, padded to NSP rows
    u0 = (dinv[:, None] * x).astype(np.float32)
    y0 = (ALPHA * dinv[:, None] * x).astype(np.float32)
    w_full = (0.9 * dinv * dinv).astype(np.float32)
    sq_full = np.sqrt(deg).astype(np.float32)

    def shard_pad2(a2d):
        out = np.zeros((C, NSP, D), dtype=np.float32)
        for c in range(C):
            out[c, :NS] = a2d[c * NS:(c + 1) * NS]
        return out

    def shard_pad1(a1d):
        out = np.zeros((C, NSP), dtype=np.float32)
        for c in range(C):
            out[c, :NS] = a1d[c * NS:(c + 1) * NS]
        return out

    u0_s = shard_pad2(u0)
    y0_s = shard_pad2(y0)
    w_s = shard_pad1(w_full)
    sq_s = shard_pad1(sq_full)

    meta = dict(T=T, NCH=NCH, mm_off=mm_off, nmm=nmm, chains=chains,
                t_off=t_off, NMM=NMM, TA=TA, TB=TB)
    return meta, idx_a_w, idx_b_w, qv16, iota16, u0_s, y0_s, w_s, sq_s


def _build_nc(meta):
    import concourse.bacc as bacc
    import concourse.mybir as mybir
    import concourse.tile as tile

    T, NCH = meta['T'], meta['NCH']
    mm_off, nmm = meta['mm_off'], meta['nmm']
    chains = meta['chains']
    t_off = meta['t_off']
    NMM, TA, TB = meta['NMM'], meta['TA'], meta['TB']

    nc = bacc.Bacc(None, target_bir_lowering=False, num_devices=C,
                   num_swdge_queues=4)
    dt = mybir.dt.float32
    bf = mybir.dt.bfloat16

    u0_d = nc.dram_tensor("u0", [NSP, D], dt, kind="ExternalInput")
    y0_d = nc.dram_tensor("y0", [NSP, D], dt, kind="ExternalInput")
    w_d = nc.dram_tensor("w", [NSP], dt, kind="ExternalInput")
    sq_d = nc.dram_tensor("sq", [NSP], dt, kind="ExternalInput")
    ia_d = nc.dram_tensor("idx_a", [128, TA // 16], mybir.dt.int16,
                          kind="ExternalInput")
    ib_d = nc.dram_tensor("idx_b", [128, TB // 16], mybir.dt.int16,
                          kind="ExternalInput")
    qv_d = nc.dram_tensor("qv", [128, NMM], bf, kind="ExternalInput")
    iota_d = nc.dram_tensor("iota", [128, 128], bf, kind="ExternalInput")
    out_d = nc.dram_tensor("out", [NSP, D], dt, kind="ExternalOutput")

    bounceA = nc.dram_tensor("bounceA", [HALFA, D], dt)
    bounceB = nc.dram_tensor("bounceB", [HALFB, D], dt)
    urepsA = [nc.dram_tensor(f"urepA{i}", [NPA, D], dt, addr_space="Shared")
              for i in range(2)]
    urepsB = [nc.dram_tensor(f"urepB{i}", [NPB, D], dt, addr_space="Shared")
              for i in range(2)]

    with tile.TileContext(nc) as tc:
        with (
            tc.tile_pool(name="res", bufs=1) as res,
            tc.tile_pool(name="mbuf", bufs=3) as mpool,
            tc.tile_pool(name="m16buf", bufs=2) as m16pool,
            tc.tile_pool(name="qbuf", bufs=2) as qpool,
            tc.tile_pool(name="psum", bufs=8, space="PSUM") as ppool,
        ):
            uA = res.tile([128, NB, D], dt, tag="uA")
            uB = res.tile([128, NB, D], dt, tag="uB")
            y0t = res.tile([128, NB, D], dt, tag="y0t")
            wt = res.tile([128, NB], dt, tag="wt")
            sqt = res.tile([128, NB], dt, tag="sqt")
            iota_t = res.tile([128, 128], bf, tag="iota")
            qv_t = res.tile([128, NMM], bf, tag="qv")
            ia_t = res.tile([128, TA // 16], mybir.dt.int16, tag="ia")
            ib_t = res.tile([128, TB // 16], mybir.dt.int16, tag="ib")

            def node_ap(dram):
                return dram[:].rearrange("(b p) f -> p b f", p=128)

            def node_ap1(dram):
                return dram[:].rearrange("(b p) -> p b", p=128)

            nc.sync.dma_start(uA[:], node_ap(u0_d))
            nc.sync.dma_start(y0t[:], node_ap(y0_d))
            nc.sync.dma_start(wt[:], node_ap1(w_d))
            nc.sync.dma_start(sqt[:], node_ap1(sq_d))
            nc.sync.dma_start(iota_t[:], iota_d[:])
            nc.sync.dma_start(qv_t[:], qv_d[:])
            nc.sync.dma_start(ia_t[:], ia_d[:])
            nc.sync.dma_start(ib_t[:], ib_d[:])

            # initial AllGather of u0 halves
            nc.sync.dma_start(bounceA[:], u0_d[0:HALFA, :])
            nc.sync.dma_start(bounceB[:], u0_d[HALFA:NSP, :])
            nc.gpsimd.collective_compute(
                "AllGather", mybir.AluOpType.bypass,
                replica_groups=[list(range(C))],
                ins=[bounceA[:]], outs=[urepsA[0][:]],
            )
            nc.gpsimd.collective_compute(
                "AllGather", mybir.AluOpType.bypass,
                replica_groups=[list(range(C))],
                ins=[bounceB[:]], outs=[urepsB[0][:]],
            )

            u_cur, u_nxt = uA, uB
            qn = [0]
            GS = 1536
            for hop in range(K):
                viewA = urepsA[hop % 2][:]
                viewB = urepsB[hop % 2][:]
                tiles = {}

                def gather_a(s):
                    ta = int(T[s, 0])
                    nca = int(NCH[s, 0])
                    sa = int(t_off[s, 0])
                    ma = mpool.tile([128, nca, D], dt, tag="ma")
                    tiles[('a', s)] = ma
                    for g0 in range(0, ta, GS):
                        g1 = min(g0 + GS, ta)
                        nc.gpsimd.dma_gather(
                            ma[:, g0 // 128:g1 // 128, :], viewA,
                            ia_t[:, (sa + g0) // 16:(sa + g1) // 16],
                            g1 - g0, g1 - g0, D, single_packet=False,
                            queue_num=qn[0] % 4)
                        qn[0] += 1

                def gather_b(s):
                    tb = int(T[s, 1])
                    ncb = int(NCH[s, 1])
                    sb = int(t_off[s, 1])
                    mb = mpool.tile([128, ncb, D], dt, tag="mb")
                    tiles[('b', s)] = mb
                    for g0 in range(0, tb, GS):
                        g1 = min(g0 + GS, tb)
                        nc.gpsimd.dma_gather(
                            mb[:, g0 // 128:g1 // 128, :], viewB,
                            ib_t[:, (sb + g0) // 16:(sb + g1) // 16],
                            g1 - g0, g1 - g0, D, single_packet=False,
                            queue_num=qn[0] % 4)
                        qn[0] += 1

                def compute(s):
                    b0, b1 = SGS[s]
                    nca = int(NCH[s, 0])
                    ncb = int(NCH[s, 1])
                    ma = tiles[('a', s)]
                    mb = tiles[('b', s)]
                    ma16 = m16pool.tile([128, nca, D], bf, tag="ma16")
                    mb16 = m16pool.tile([128, ncb, D], bf, tag="mb16")
                    nc.scalar.copy(out=ma16[:], in_=ma[:])
                    nc.scalar.copy(out=mb16[:], in_=mb[:])

                    # on-chip one-hot generation for this supergroup
                    na = int(nmm[s, 0])
                    nb_ = int(nmm[s, 1])
                    oa = int(mm_off[s, 0])
                    ob = int(mm_off[s, 1])
                    qa = qpool.tile([128, na, 128], bf, tag="qa")
                    qb = qpool.tile([128, nb_, 128], bf, tag="qb")
                    nc.vector.tensor_tensor(
                        out=qa[:],
                        in0=qv_t[:, oa:oa + na].unsqueeze(2)
                            .broadcast_to([128, na, 128]),
                        in1=iota_t[:].unsqueeze(1)
                            .broadcast_to([128, na, 128]),
                        op=mybir.AluOpType.is_equal)
                    nc.vector.tensor_tensor(
                        out=qb[:],
                        in0=qv_t[:, ob:ob + nb_].unsqueeze(2)
                            .broadcast_to([128, nb_, 128]),
                        in1=iota_t[:].unsqueeze(1)
                            .broadcast_to([128, nb_, 128]),
                        op=mybir.AluOpType.is_equal)

                    for b in range(b0, b1):
                        chain = chains[s][b]
                        ps = ppool.tile([128, D], dt, tag="ps")
                        tot = len(chain)
                        for j, (h, k, col) in enumerate(chain):
                            qt = qa if h == 0 else qb
                            mt = ma16 if h == 0 else mb16
                            nc.tensor.matmul(
                                ps[:], qt[:, col, :], mt[:, k, :],
                                start=(j == 0), stop=(j == tot - 1))
                        # u_new = w * (agg + u) + y0
                        nc.vector.tensor_tensor(
                            out=u_nxt[:, b, :], in0=ps[:], in1=u_cur[:, b, :],
                            op=mybir.AluOpType.add)
                        # (two tensor_tensor ops: the tensor_scalar family can
                        # enter 2-port DVE perf mode, which locks GpSimd out
                        # of the shared SBUF port mid-gather-gen)
                        nc.vector.tensor_tensor(
                            out=u_nxt[:, b, :], in0=u_nxt[:, b, :],
                            in1=wt[:, b:b + 1].broadcast_to([128, D]),
                            op=mybir.AluOpType.mult)
                        nc.vector.tensor_tensor(
                            out=u_nxt[:, b, :], in0=u_nxt[:, b, :],
                            in1=y0t[:, b, :],
                            op=mybir.AluOpType.add)

                    if hop < K - 1:
                        if b1 <= NBA:
                            nc.sync.dma_start(
                                node_ap(bounceA)[:, b0:b1, :],
                                u_nxt[:, b0:b1, :])
                        else:
                            nc.sync.dma_start(
                                node_ap(bounceB)[:, b0 - NBA:b1 - NBA, :],
                                u_nxt[:, b0:b1, :])
                        if b1 == NBA:
                            # blocks 0-24 done: fire AG-A mid-hop
                            nc.gpsimd.collective_compute(
                                "AllGather", mybir.AluOpType.bypass,
                                replica_groups=[list(range(C))],
                                ins=[bounceA[:]],
                                outs=[urepsA[(hop + 1) % 2][:]],
                            )

                # Stagger B-stream gathers two supergroups behind A-stream:
                # B-gathers wait on AG-B of the previous hop, and the GpSimd
                # engine FIFO is in-order, so a blocked B-gather must not sit
                # in front of ready A-gathers.
                LAG = 2
                for s in range(NSG + LAG):
                    if s < NSG:
                        gather_a(s)
                    if s >= LAG:
                        gather_b(s - LAG)
                        compute(s - LAG)

                if hop < K - 1:
                    nc.gpsimd.collective_compute(
                        "AllGather", mybir.AluOpType.bypass,
                        replica_groups=[list(range(C))],
                        ins=[bounceB[:]], outs=[urepsB[(hop + 1) % 2][:]],
                    )
                u_cur, u_nxt = u_nxt, u_cur

            # epilogue: out = relu(u * sqrt(deg))
            ot = u_nxt  # reuse the dead double buffer
            nc.vector.tensor_tensor(
                out=ot[:], in0=u_cur[:],
                in1=sqt[:].unsqueeze(2).broadcast_to([128, NB, D]),
                op=mybir.AluOpType.mult)
            nc.vector.tensor_scalar_max(out=ot[:], in0=ot[:], scalar1=0.0)
            nc.sync.dma_start(node_ap(out_d), ot[:])

    nc.compile()
    return nc


def kernel(x, edge_index):
    (meta, idx_a_w, idx_b_w, qv16, iota16,
     u0_s, y0_s, w_s, sq_s) = _host_prep(x, edge_index)
    nc = _build_nc(meta)

    from concourse.bass_utils import run_bass_kernel_spmd

    in_maps = []
    for c in range(C):
        in_maps.append({
            "u0": u0_s[c], "y0": y0_s[c], "w": w_s[c], "sq": sq_s[c],
            "idx_a": idx_a_w[c], "idx_b": idx_b_w[c],
            "qv": qv16[c], "iota": iota16,
        })

    ntff_dir = os.environ.get("APPNP_NTFF_DIR")
    if ntff_dir:
        from trn_agent_boot.trn_boot import _ntff_profile_via_ctypes
        hook = _ntff_profile_via_ctypes('/opt/axon/libaxon_pjrt.so')
        os.makedirs(ntff_dir, exist_ok=True)
        with hook(ntff_dir, None):
            res = run_bass_kernel_spmd(nc, in_maps, core_ids=list(range(C)))
    else:
        res = run_bass_kernel_spmd(nc, in_maps, core_ids=list(range(C)))

    out = np.empty((N, D), dtype=np.float32)
    for c in range(C):
        out[c * NS:(c + 1) * NS] = res.results[c]["out"][:NS]
    return out


# revision 13
# speedup vs baseline: 1.0440x; 1.0010x over previous
"""APPNP (10-hop propagation) on 8 TRN2 NeuronCores.

Strategy: shard destination nodes across the 8 cores (6250 each, padded to
6272 = 49*128). Per hop and per core: dma_gather the messages u[src] for the
core's in-edges (tokens pre-sorted by dst on the host), segment-sum them on
the TensorEngine via one-hot matrices generated ON-CHIP (DVE is_equal of a
preloaded per-token dst vector against an iota row), apply the teleport
update, and AllGather the new u shards into replicated HBM tables for the
next hop's gather.

Each shard is split at row 3200 into an A half (rows 0:3200, blocks 0-24)
and a B half (rows 3200:6272, blocks 25-48). The halves are AllGathered
separately: AG-A fires mid-hop (right after blocks 0-24 are computed) and
overlaps the second half of the hop; AG-B fires at hop end and hides under
the next hop's A-stream gathers (which only depend on AG-A). Both gathered
tables (25600 and 24576 rows) are addressable with int16 indices, so no
lo/hi index split is needed. Tokens are packed densely; chunks that straddle
a dst-block boundary issue one extra matmul whose Q columns auto-mask via
the range compare.
"""
import os
import sys

sys.path.insert(0, '/opt/trn_rl_repo')

import numpy as np

N = 50000
D = 64
E = 800000
K = 10
ALPHA = 0.1
C = 8                 # cores
NS = 6250             # real dst nodes per core
NSP = 6272            # padded (49 * 128)
NB = 49               # dst blocks per core
HALFA = 3200          # rows per core in the A half (blocks 0-24)
HALFB = NSP - HALFA   # 3072 rows, blocks 25-48
NBA = HALFA // 128    # 25
NPA = C * HALFA       # 25600 rows in table A
NPB = C * HALFB       # 24576 rows in table B
SGB = 5               # dst blocks per supergroup
SGS = [(b, min(b + SGB, NB)) for b in range(0, NB, SGB)]
NSG = len(SGS)


def _host_prep(x, edge_index):
    import ml_dtypes

    src = np.asarray(edge_index[0], dtype=np.int64)
    dst = np.asarray(edge_index[1], dtype=np.int64)
    x = np.asarray(x, dtype=np.float32)

    deg = np.bincount(dst, minlength=N).astype(np.float64) + 1.0
    dinv = 1.0 / np.sqrt(deg)

    src_core = src // NS
    src_row = src % NS
    # half-table row index of each edge's source
    is_a = src_row < HALFA
    rowA = src_core * HALFA + src_row
    rowB = src_core * HALFB + (src_row - HALFA)

    core = dst // NS
    dst_local = dst - core * NS

    # ---- per (core, supergroup, half) dst-sorted token streams ----
    toks = {}   # (c, s, h) -> (table_row int64, dstloc int64)
    for c in range(C):
        mc = core == c
        ra_c = rowA[mc]
        rb_c = rowB[mc]
        dl_c = dst_local[mc]
        a_c = is_a[mc]
        blk = dl_c // 128
        for s, (b0, b1) in enumerate(SGS):
            ms = (blk >= b0) & (blk < b1)
            for h in range(2):
                mh = ms & (a_c if h == 0 else ~a_c)
                rr = (ra_c if h == 0 else rb_c)[mh]
                dl = dl_c[mh]
                order = np.argsort(dl, kind='stable')
                toks[(c, s, h)] = (rr[order], dl[order])

    # static (shared across cores) stream sizes, rounded to 128
    T = np.zeros((NSG, 2), dtype=np.int64)
    for s in range(NSG):
        for h in range(2):
            mx = max(len(toks[(c, s, h)][0]) for c in range(C))
            T[s, h] = ((mx + 127) // 128) * 128
    NCH = T // 128

    # ---- static MM instance table ----
    mm_off = np.zeros((NSG, 2), dtype=np.int64)
    nmm = np.zeros((NSG, 2), dtype=np.int64)
    chains = []        # per sg: dict b -> list[(h, ch, local_col)]
    inst_list = []     # per (s, h): list of (ch, b) in col order
    off = 0
    for s, (b0, b1) in enumerate(SGS):
        ch_map = {b: [] for b in range(b0, b1)}
        per_sh = []
        for h in range(2):
            mm_off[s, h] = off
            insts = []
            for k in range(NCH[s, h]):
                blocks = set()
                for c in range(C):
                    dl = toks[(c, s, h)][1]
                    seg = dl[k * 128:(k + 1) * 128]
                    if len(seg):
                        blocks.update((seg // 128).tolist())
                for b in sorted(blocks):
                    insts.append((k, b))
            for j, (k, b) in enumerate(insts):
                ch_map[b].append((h, k, j))
            nmm[s, h] = len(insts)
            off += len(insts)
            per_sh.append(insts)
        chains.append(ch_map)
        inst_list.append(per_sh)
    NMM = int(off)

    # ---- per-core qv + idx arrays ----
    qv = np.full((C, 128, NMM), -1.0, dtype=np.float32)
    TA = int(T[:, 0].sum())
    TB = int(T[:, 1].sum())
    idx_a = np.zeros((C, TA), dtype=np.int16)
    idx_b = np.zeros((C, TB), dtype=np.int16)
    t_off = np.zeros((NSG, 2), dtype=np.int64)
    oa = 0
    ob = 0
    for s in range(NSG):
        t_off[s, 0] = oa
        t_off[s, 1] = ob
        oa += int(T[s, 0])
        ob += int(T[s, 1])

    for c in range(C):
        for s in range(NSG):
            for h in range(2):
                rr, dl = toks[(c, s, h)]
                n = len(rr)
                o = int(t_off[s, h])
                if h == 0:
                    idx_a[c, o:o + n] = rr.astype(np.int16)
                else:
                    idx_b[c, o:o + n] = rr.astype(np.int16)
                # pads stay 0 (fetch row 0; Q col is zero)
                dpad = np.full(int(T[s, h]), -1.0, dtype=np.float32)
                dpad[:n] = dl.astype(np.float32)
                for j, (k, b) in enumerate(inst_list[s][h]):
                    col = int(mm_off[s, h]) + j
                    seg = dpad[k * 128:(k + 1) * 128]
                    qv[c, :len(seg), col] = seg - 128.0 * b

    qv16 = qv.astype(ml_dtypes.bfloat16)

    def wrap16(a):
        n = a.shape[-1]
        w = a.reshape(C, n // 16, 16).transpose(0, 2, 1)
        return np.tile(w, (1, 8, 1)).copy()

    idx_a_w = wrap16(idx_a)
    idx_b_w = wrap16(idx_b)

    iota = np.broadcast_to(np.arange(128, dtype=np.float32), (128, 128))
    iota16 = np.ascontiguousarray(iota).astype(ml_dtypes.bfloat16)

    # per-core node-level tensors, padded to NSP rows
    u0 = (dinv[:, None] * x).astype(np.float32)
    y0 = (ALPHA * dinv[:, None] * x).astype(np.float32)
    w_full = (0.9 * dinv * dinv).astype(np.float32)
    sq_full = np.sqrt(deg).astype(np.float32)

    def shard_pad2(a2d):
        out = np.zeros((C, NSP, D), dtype=np.float32)
        for c in range(C):
            out[c, :NS] = a2d[c * NS:(c + 1) * NS]
        return out

    def shard_pad1(a1d):
        out = np.zeros((C, NSP), dtype=np.float32)
        for c in range(C):
            out[c, :NS] = a1d[c * NS:(c + 1) * NS]
        return out

    u0_s = shard_pad2(u0)
    y0_s = shard_pad2(y0)
    w_s = shard_pad1(w_full)
    sq_s = shard_pad1(sq_full)

    meta = dict(T=T, NCH=NCH, mm_off=mm_off, nmm=nmm, chains=chains,
                t_off=t_off, NMM=NMM, TA=TA, TB=TB)
    return meta, idx_a_w, idx_b_w, qv16, iota16, u0_s, y0_s, w_s, sq_s


def _build_nc(meta):
    import concourse.bacc as bacc
    import concourse.mybir as mybir
    import concourse.tile as tile

    T, NCH = meta['T'], meta['NCH']
    mm_off, nmm = meta['mm_off'], meta['nmm']
    chains = meta['chains']
    t_off = meta['t_off']
    NMM, TA, TB = meta['NMM'], meta['TA'], meta['TB']

    nc = bacc.Bacc(None, target_bir_lowering=False, num_devices=C,
                   num_swdge_queues=4)
    dt = mybir.dt.float32
    bf = mybir.dt.bfloat16

    u0_d = nc.dram_tensor("u0", [NSP, D], dt, kind="ExternalInput")
    y0_d = nc.dram_tensor("y0", [NSP, D], dt, kind="ExternalInput")
    w_d = nc.dram_tensor("w", [NSP], dt, kind="ExternalInput")
    sq_d = nc.dram_tensor("sq", [NSP], dt, kind="ExternalInput")
    ia_d = nc.dram_tensor("idx_a", [128, TA // 16], mybir.dt.int16,
                          kind="ExternalInput")
    ib_d = nc.dram_tensor("idx_b", [128, TB // 16], mybir.dt.int16,
                          kind="ExternalInput")
    qv_d = nc.dram_tensor("qv", [128, NMM], bf, kind="ExternalInput")
    iota_d = nc.dram_tensor("iota", [128, 128], bf, kind="ExternalInput")
    out_d = nc.dram_tensor("out", [NSP, D], dt, kind="ExternalOutput")

    bounceA = nc.dram_tensor("bounceA", [HALFA, D], dt)
    bounceB = nc.dram_tensor("bounceB", [HALFB, D], dt)
    urepsA = [nc.dram_tensor(f"urepA{i}", [NPA, D], dt, addr_space="Shared")
              for i in range(2)]
    urepsB = [nc.dram_tensor(f"urepB{i}", [NPB, D], dt, addr_space="Shared")
              for i in range(2)]

    with tile.TileContext(nc) as tc:
        with (
            tc.tile_pool(name="res", bufs=1) as res,
            tc.tile_pool(name="mbufA", bufs=4) as mpoolA,
            tc.tile_pool(name="mbufB", bufs=3) as mpoolB,
            tc.tile_pool(name="m16buf", bufs=2) as m16pool,
            tc.tile_pool(name="qbuf", bufs=2) as qpool,
            tc.tile_pool(name="psum", bufs=8, space="PSUM") as ppool,
        ):
            uA = res.tile([128, NB, D], dt, tag="uA")
            uB = res.tile([128, NB, D], dt, tag="uB")
            y0t = res.tile([128, NB, D], dt, tag="y0t")
            wt = res.tile([128, NB], dt, tag="wt")
            sqt = res.tile([128, NB], dt, tag="sqt")
            iota_t = res.tile([128, 128], bf, tag="iota")
            qv_t = res.tile([128, NMM], bf, tag="qv")
            ia_t = res.tile([128, TA // 16], mybir.dt.int16, tag="ia")
            ib_t = res.tile([128, TB // 16], mybir.dt.int16, tag="ib")

            def node_ap(dram):
                return dram[:].rearrange("(b p) f -> p b f", p=128)

            def node_ap1(dram):
                return dram[:].rearrange("(b p) -> p b", p=128)

            nc.sync.dma_start(uA[:], node_ap(u0_d))
            nc.sync.dma_start(y0t[:], node_ap(y0_d))
            nc.sync.dma_start(wt[:], node_ap1(w_d))
            nc.sync.dma_start(sqt[:], node_ap1(sq_d))
            nc.sync.dma_start(iota_t[:], iota_d[:])
            nc.sync.dma_start(qv_t[:], qv_d[:])
            nc.sync.dma_start(ia_t[:], ia_d[:])
            nc.sync.dma_start(ib_t[:], ib_d[:])

            # initial AllGather of u0 halves
            nc.sync.dma_start(bounceA[:], u0_d[0:HALFA, :])
            nc.sync.dma_start(bounceB[:], u0_d[HALFA:NSP, :])
            nc.gpsimd.collective_compute(
                "AllGather", mybir.AluOpType.bypass,
                replica_groups=[list(range(C))],
                ins=[bounceA[:]], outs=[urepsA[0][:]],
            )
            nc.gpsimd.collective_compute(
                "AllGather", mybir.AluOpType.bypass,
                replica_groups=[list(range(C))],
                ins=[bounceB[:]], outs=[urepsB[0][:]],
            )

            u_cur, u_nxt = uA, uB
            qn = [0]
            GS = 1536
            for hop in range(K):
                viewA = urepsA[hop % 2][:]
                viewB = urepsB[hop % 2][:]
                tiles = {}

                def gather_a(s):
                    ta = int(T[s, 0])
                    nca = int(NCH[s, 0])
                    sa = int(t_off[s, 0])
                    ma = mpoolA.tile([128, nca, D], dt, tag="ma")
                    tiles[('a', s)] = ma
                    for g0 in range(0, ta, GS):
                        g1 = min(g0 + GS, ta)
                        nc.gpsimd.dma_gather(
                            ma[:, g0 // 128:g1 // 128, :], viewA,
                            ia_t[:, (sa + g0) // 16:(sa + g1) // 16],
                            g1 - g0, g1 - g0, D, single_packet=False,
                            queue_num=qn[0] % 4)
                        qn[0] += 1

                def gather_b(s):
                    tb = int(T[s, 1])
                    ncb = int(NCH[s, 1])
                    sb = int(t_off[s, 1])
                    mb = mpoolB.tile([128, ncb, D], dt, tag="mb")
                    tiles[('b', s)] = mb
                    for g0 in range(0, tb, GS):
                        g1 = min(g0 + GS, tb)
                        nc.gpsimd.dma_gather(
                            mb[:, g0 // 128:g1 // 128, :], viewB,
                            ib_t[:, (sb + g0) // 16:(sb + g1) // 16],
                            g1 - g0, g1 - g0, D, single_packet=False,
                            queue_num=qn[0] % 4)
                        qn[0] += 1

                def compute(s):
                    b0, b1 = SGS[s]
                    nca = int(NCH[s, 0])
                    ncb = int(NCH[s, 1])
                    ma = tiles[('a', s)]
                    mb = tiles[('b', s)]
                    ma16 = m16pool.tile([128, nca, D], bf, tag="ma16")
                    mb16 = m16pool.tile([128, ncb, D], bf, tag="mb16")
                    nc.scalar.copy(out=ma16[:], in_=ma[:])
                    nc.scalar.copy(out=mb16[:], in_=mb[:])

                    # on-chip one-hot generation for this supergroup
                    na = int(nmm[s, 0])
                    nb_ = int(nmm[s, 1])
                    oa = int(mm_off[s, 0])
                    ob = int(mm_off[s, 1])
                    qa = qpool.tile([128, na, 128], bf, tag="qa")
                    qb = qpool.tile([128, nb_, 128], bf, tag="qb")
                    nc.vector.tensor_tensor(
                        out=qa[:],
                        in0=qv_t[:, oa:oa + na].unsqueeze(2)
                            .broadcast_to([128, na, 128]),
                        in1=iota_t[:].unsqueeze(1)
                            .broadcast_to([128, na, 128]),
                        op=mybir.AluOpType.is_equal)
                    nc.vector.tensor_tensor(
                        out=qb[:],
                        in0=qv_t[:, ob:ob + nb_].unsqueeze(2)
                            .broadcast_to([128, nb_, 128]),
                        in1=iota_t[:].unsqueeze(1)
                            .broadcast_to([128, nb_, 128]),
                        op=mybir.AluOpType.is_equal)

                    for b in range(b0, b1):
                        chain = chains[s][b]
                        ps = ppool.tile([128, D], dt, tag="ps")
                        tot = len(chain)
                        for j, (h, k, col) in enumerate(chain):
                            qt = qa if h == 0 else qb
                            mt = ma16 if h == 0 else mb16
                            nc.tensor.matmul(
                                ps[:], qt[:, col, :], mt[:, k, :],
                                start=(j == 0), stop=(j == tot - 1))
                        # u_new = w * (agg + u) + y0
                        nc.vector.tensor_tensor(
                            out=u_nxt[:, b, :], in0=ps[:], in1=u_cur[:, b, :],
                            op=mybir.AluOpType.add)
                        # (two tensor_tensor ops: the tensor_scalar family can
                        # enter 2-port DVE perf mode, which locks GpSimd out
                        # of the shared SBUF port mid-gather-gen)
                        nc.vector.tensor_tensor(
                            out=u_nxt[:, b, :], in0=u_nxt[:, b, :],
                            in1=wt[:, b:b + 1].broadcast_to([128, D]),
                            op=mybir.AluOpType.mult)
                        nc.vector.tensor_tensor(
                            out=u_nxt[:, b, :], in0=u_nxt[:, b, :],
                            in1=y0t[:, b, :],
                            op=mybir.AluOpType.add)

                    if hop < K - 1:
                        if b1 <= NBA:
                            nc.sync.dma_start(
                                node_ap(bounceA)[:, b0:b1, :],
                                u_nxt[:, b0:b1, :])
                        else:
                            nc.sync.dma_start(
                                node_ap(bounceB)[:, b0 - NBA:b1 - NBA, :],
                                u_nxt[:, b0:b1, :])
                        if b1 == NBA:
                            # blocks 0-24 done: fire AG-A mid-hop
                            nc.gpsimd.collective_compute(
                                "AllGather", mybir.AluOpType.bypass,
                                replica_groups=[list(range(C))],
                                ins=[bounceA[:]],
                                outs=[urepsA[(hop + 1) % 2][:]],
                            )

                # Stagger B-stream gathers two supergroups behind A-stream:
                # B-gathers wait on AG-B of the previous hop, and the GpSimd
                # engine FIFO is in-order, so a blocked B-gather must not sit
                # in front of ready A-gathers.
                LAG = 2
                for s in range(NSG + LAG):
                    if s < NSG:
                        gather_a(s)
                    if s >= LAG:
                        gather_b(s - LAG)
                        compute(s - LAG)

                if hop < K - 1:
                    nc.gpsimd.collective_compute(
                        "AllGather", mybir.AluOpType.bypass,
                        replica_groups=[list(range(C))],
                        ins=[bounceB[:]], outs=[urepsB[(hop + 1) % 2][:]],
                    )
                u_cur, u_nxt = u_nxt, u_cur

            # epilogue: out = relu(u * sqrt(deg))
            ot = u_nxt  # reuse the dead double buffer
            nc.vector.tensor_tensor(
                out=ot[:], in0=u_cur[:],
                in1=sqt[:].unsqueeze(2).broadcast_to([128, NB, D]),
                op=mybir.AluOpType.mult)
            nc.vector.tensor_scalar_max(out=ot[:], in0=ot[:], scalar1=0.0)
            nc.sync.dma_start(node_ap(out_d), ot[:])

    nc.compile()
    return nc


def kernel(x, edge_index):
    (meta, idx_a_w, idx_b_w, qv16, iota16,
     u0_s, y0_s, w_s, sq_s) = _host_prep(x, edge_index)
    nc = _build_nc(meta)

    from concourse.bass_utils import run_bass_kernel_spmd

    in_maps = []
    for c in range(C):
        in_maps.append({
            "u0": u0_s[c], "y0": y0_s[c], "w": w_s[c], "sq": sq_s[c],
            "idx_a": idx_a_w[c], "idx_b": idx_b_w[c],
            "qv": qv16[c], "iota": iota16,
        })

    ntff_dir = os.environ.get("APPNP_NTFF_DIR")
    if ntff_dir:
        from trn_agent_boot.trn_boot import _ntff_profile_via_ctypes
        hook = _ntff_profile_via_ctypes('/opt/axon/libaxon_pjrt.so')
        os.makedirs(ntff_dir, exist_ok=True)
        with hook(ntff_dir, None):
            res = run_bass_kernel_spmd(nc, in_maps, core_ids=list(range(C)))
    else:
        res = run_bass_kernel_spmd(nc, in_maps, core_ids=list(range(C)))

    out = np.empty((N, D), dtype=np.float32)
    for c in range(C):
        out[c * NS:(c + 1) * NS] = res.results[c]["out"][:NS]
    return out


# revision 14
# speedup vs baseline: 1.0617x; 1.0169x over previous
"""APPNP (10-hop propagation) on 8 TRN2 NeuronCores.

Strategy: shard destination nodes across the 8 cores (6250 each, padded to
6272 = 49*128). Per hop and per core: dma_gather the messages u[src] for the
core's in-edges (tokens pre-sorted by dst on the host), segment-sum them on
the TensorEngine via one-hot matrices generated ON-CHIP (DVE is_equal of a
preloaded per-token dst vector against an iota row), apply the teleport
update, and AllGather the new u shards into replicated HBM tables for the
next hop's gather.

Each shard is split at row 3200 into an A half (rows 0:3200, blocks 0-24)
and a B half (rows 3200:6272, blocks 25-48). The halves are AllGathered
separately: AG-A fires mid-hop (right after blocks 0-24 are computed) and
overlaps the second half of the hop; AG-B fires at hop end and hides under
the next hop's A-stream gathers (which only depend on AG-A). Both gathered
tables (25600 and 24576 rows) are addressable with int16 indices, so no
lo/hi index split is needed. Tokens are packed densely; chunks that straddle
a dst-block boundary issue one extra matmul whose Q columns auto-mask via
the range compare.
"""
import os
import sys

sys.path.insert(0, '/opt/trn_rl_repo')

import numpy as np

N = 50000
D = 64
E = 800000
K = 10
ALPHA = 0.1
C = 8                 # cores
NS = 6250             # real dst nodes per core
NSP = 6272            # padded (49 * 128)
NB = 49               # dst blocks per core
HALFA = 3200          # rows per core in the A half (blocks 0-24)
HALFB = NSP - HALFA   # 3072 rows, blocks 25-48
NBA = HALFA // 128    # 25
NPA = C * HALFA       # 25600 rows in table A
NPB = C * HALFB       # 24576 rows in table B
SGB = 5               # dst blocks per supergroup
SGS = [(b, min(b + SGB, NB)) for b in range(0, NB, SGB)]
NSG = len(SGS)


def _host_prep(x, edge_index):
    import ml_dtypes

    src = np.asarray(edge_index[0], dtype=np.int64)
    dst = np.asarray(edge_index[1], dtype=np.int64)
    x = np.asarray(x, dtype=np.float32)

    deg = np.bincount(dst, minlength=N).astype(np.float64) + 1.0
    dinv = 1.0 / np.sqrt(deg)

    src_core = src // NS
    src_row = src % NS
    # half-table row index of each edge's source
    is_a = src_row < HALFA
    rowA = src_core * HALFA + src_row
    rowB = src_core * HALFB + (src_row - HALFA)

    core = dst // NS
    dst_local = dst - core * NS

    # ---- per (core, supergroup, half) dst-sorted token streams ----
    toks = {}   # (c, s, h) -> (table_row int64, dstloc int64)
    for c in range(C):
        mc = core == c
        ra_c = rowA[mc]
        rb_c = rowB[mc]
        dl_c = dst_local[mc]
        a_c = is_a[mc]
        blk = dl_c // 128
        for s, (b0, b1) in enumerate(SGS):
            ms = (blk >= b0) & (blk < b1)
            for h in range(2):
                mh = ms & (a_c if h == 0 else ~a_c)
                rr = (ra_c if h == 0 else rb_c)[mh]
                dl = dl_c[mh]
                order = np.argsort(dl, kind='stable')
                toks[(c, s, h)] = (rr[order], dl[order])

    # static (shared across cores) stream sizes, rounded to 128
    T = np.zeros((NSG, 2), dtype=np.int64)
    for s in range(NSG):
        for h in range(2):
            mx = max(len(toks[(c, s, h)][0]) for c in range(C))
            T[s, h] = ((mx + 127) // 128) * 128
    NCH = T // 128

    # ---- static MM instance table ----
    mm_off = np.zeros((NSG, 2), dtype=np.int64)
    nmm = np.zeros((NSG, 2), dtype=np.int64)
    chains = []        # per sg: dict b -> list[(h, ch, local_col)]
    inst_list = []     # per (s, h): list of (ch, b) in col order
    off = 0
    for s, (b0, b1) in enumerate(SGS):
        ch_map = {b: [] for b in range(b0, b1)}
        per_sh = []
        for h in range(2):
            mm_off[s, h] = off
            insts = []
            for k in range(NCH[s, h]):
                blocks = set()
                for c in range(C):
                    dl = toks[(c, s, h)][1]
                    seg = dl[k * 128:(k + 1) * 128]
                    if len(seg):
                        blocks.update((seg // 128).tolist())
                for b in sorted(blocks):
                    insts.append((k, b))
            for j, (k, b) in enumerate(insts):
                ch_map[b].append((h, k, j))
            nmm[s, h] = len(insts)
            off += len(insts)
            per_sh.append(insts)
        chains.append(ch_map)
        inst_list.append(per_sh)
    NMM = int(off)

    # ---- per-core qv + idx arrays ----
    qv = np.full((C, 128, NMM), -1.0, dtype=np.float32)
    TA = int(T[:, 0].sum())
    TB = int(T[:, 1].sum())
    idx_a = np.zeros((C, TA), dtype=np.int16)
    idx_b = np.zeros((C, TB), dtype=np.int16)
    t_off = np.zeros((NSG, 2), dtype=np.int64)
    oa = 0
    ob = 0
    for s in range(NSG):
        t_off[s, 0] = oa
        t_off[s, 1] = ob
        oa += int(T[s, 0])
        ob += int(T[s, 1])

    for c in range(C):
        for s in range(NSG):
            for h in range(2):
                rr, dl = toks[(c, s, h)]
                n = len(rr)
                o = int(t_off[s, h])
                if h == 0:
                    idx_a[c, o:o + n] = rr.astype(np.int16)
                else:
                    idx_b[c, o:o + n] = rr.astype(np.int16)
                # pads stay 0 (fetch row 0; Q col is zero)
                dpad = np.full(int(T[s, h]), -1.0, dtype=np.float32)
                dpad[:n] = dl.astype(np.float32)
                for j, (k, b) in enumerate(inst_list[s][h]):
                    col = int(mm_off[s, h]) + j
                    seg = dpad[k * 128:(k + 1) * 128]
                    qv[c, :len(seg), col] = seg - 128.0 * b

    qv16 = qv.astype(ml_dtypes.bfloat16)

    def wrap16(a):
        n = a.shape[-1]
        w = a.reshape(C, n // 16, 16).transpose(0, 2, 1)
        return np.tile(w, (1, 8, 1)).copy()

    idx_a_w = wrap16(idx_a)
    idx_b_w = wrap16(idx_b)

    iota = np.broadcast_to(np.arange(128, dtype=np.float32), (128, 128))
    iota16 = np.ascontiguousarray(iota).astype(ml_dtypes.bfloat16)

    # per-core node-level tensors, padded to NSP rows
    u0 = (dinv[:, None] * x).astype(np.float32)
    y0 = (ALPHA * dinv[:, None] * x).astype(np.float32)
    w_full = (0.9 * dinv * dinv).astype(np.float32)
    sq_full = np.sqrt(deg).astype(np.float32)

    def shard_pad2(a2d):
        out = np.zeros((C, NSP, D), dtype=np.float32)
        for c in range(C):
            out[c, :NS] = a2d[c * NS:(c + 1) * NS]
        return out

    def shard_pad1(a1d):
        out = np.zeros((C, NSP), dtype=np.float32)
        for c in range(C):
            out[c, :NS] = a1d[c * NS:(c + 1) * NS]
        return out

    u0_s = shard_pad2(u0)
    y0_s = shard_pad2(y0)
    w_s = shard_pad1(w_full)
    sq_s = shard_pad1(sq_full)

    meta = dict(T=T, NCH=NCH, mm_off=mm_off, nmm=nmm, chains=chains,
                t_off=t_off, NMM=NMM, TA=TA, TB=TB)
    return meta, idx_a_w, idx_b_w, qv16, iota16, u0_s, y0_s, w_s, sq_s


def _build_nc(meta):
    import concourse.bacc as bacc
    import concourse.mybir as mybir
    import concourse.tile as tile

    T, NCH = meta['T'], meta['NCH']
    mm_off, nmm = meta['mm_off'], meta['nmm']
    chains = meta['chains']
    t_off = meta['t_off']
    NMM, TA, TB = meta['NMM'], meta['TA'], meta['TB']

    nc = bacc.Bacc(None, target_bir_lowering=False, num_devices=C,
                   num_swdge_queues=4)
    dt = mybir.dt.float32
    bf = mybir.dt.bfloat16

    u0_d = nc.dram_tensor("u0", [NSP, D], dt, kind="ExternalInput")
    y0_d = nc.dram_tensor("y0", [NSP, D], dt, kind="ExternalInput")
    w_d = nc.dram_tensor("w", [NSP], dt, kind="ExternalInput")
    sq_d = nc.dram_tensor("sq", [NSP], dt, kind="ExternalInput")
    ia_d = nc.dram_tensor("idx_a", [128, TA // 16], mybir.dt.int16,
                          kind="ExternalInput")
    ib_d = nc.dram_tensor("idx_b", [128, TB // 16], mybir.dt.int16,
                          kind="ExternalInput")
    qv_d = nc.dram_tensor("qv", [128, NMM], bf, kind="ExternalInput")
    iota_d = nc.dram_tensor("iota", [128, 128], bf, kind="ExternalInput")
    out_d = nc.dram_tensor("out", [NSP, D], dt, kind="ExternalOutput")

    bounceA = nc.dram_tensor("bounceA", [HALFA, D], dt)
    bounceB = nc.dram_tensor("bounceB", [HALFB, D], dt)
    urepsA = [nc.dram_tensor(f"urepA{i}", [NPA, D], dt, addr_space="Shared")
              for i in range(2)]
    urepsB = [nc.dram_tensor(f"urepB{i}", [NPB, D], dt, addr_space="Shared")
              for i in range(2)]

    with tile.TileContext(nc) as tc:
        with (
            tc.tile_pool(name="res", bufs=1) as res,
            tc.tile_pool(name="mbufA", bufs=4) as mpoolA,
            tc.tile_pool(name="mbufB", bufs=3) as mpoolB,
            tc.tile_pool(name="m16buf", bufs=2) as m16pool,
            tc.tile_pool(name="qbuf", bufs=2) as qpool,
            tc.tile_pool(name="psum", bufs=8, space="PSUM") as ppool,
        ):
            uA = res.tile([128, NB, D], dt, tag="uA")
            uB = res.tile([128, NB, D], dt, tag="uB")
            y0t = res.tile([128, NB, D], dt, tag="y0t")
            wt = res.tile([128, NB], dt, tag="wt")
            sqt = res.tile([128, NB], dt, tag="sqt")
            iota_t = res.tile([128, 128], bf, tag="iota")
            qv_t = res.tile([128, NMM], bf, tag="qv")
            ia_t = res.tile([128, TA // 16], mybir.dt.int16, tag="ia")
            ib_t = res.tile([128, TB // 16], mybir.dt.int16, tag="ib")

            def node_ap(dram):
                return dram[:].rearrange("(b p) f -> p b f", p=128)

            def node_ap1(dram):
                return dram[:].rearrange("(b p) -> p b", p=128)

            nc.sync.dma_start(uA[:], node_ap(u0_d))
            nc.sync.dma_start(y0t[:], node_ap(y0_d))
            nc.sync.dma_start(wt[:], node_ap1(w_d))
            nc.sync.dma_start(sqt[:], node_ap1(sq_d))
            nc.sync.dma_start(iota_t[:], iota_d[:])
            nc.sync.dma_start(qv_t[:], qv_d[:])
            nc.sync.dma_start(ia_t[:], ia_d[:])
            nc.sync.dma_start(ib_t[:], ib_d[:])

            # initial AllGather of u0 halves
            nc.sync.dma_start(bounceA[:], u0_d[0:HALFA, :])
            nc.sync.dma_start(bounceB[:], u0_d[HALFA:NSP, :])
            nc.gpsimd.collective_compute(
                "AllGather", mybir.AluOpType.bypass,
                replica_groups=[list(range(C))],
                ins=[bounceA[:]], outs=[urepsA[0][:]],
            )
            nc.gpsimd.collective_compute(
                "AllGather", mybir.AluOpType.bypass,
                replica_groups=[list(range(C))],
                ins=[bounceB[:]], outs=[urepsB[0][:]],
            )

            u_cur, u_nxt = uA, uB
            qn = [0]
            GS = 2048
            for hop in range(K):
                viewA = urepsA[hop % 2][:]
                viewB = urepsB[hop % 2][:]
                tiles = {}

                def gather_a(s):
                    ta = int(T[s, 0])
                    nca = int(NCH[s, 0])
                    sa = int(t_off[s, 0])
                    ma = mpoolA.tile([128, nca, D], dt, tag="ma")
                    tiles[('a', s)] = ma
                    for g0 in range(0, ta, GS):
                        g1 = min(g0 + GS, ta)
                        nc.gpsimd.dma_gather(
                            ma[:, g0 // 128:g1 // 128, :], viewA,
                            ia_t[:, (sa + g0) // 16:(sa + g1) // 16],
                            g1 - g0, g1 - g0, D, single_packet=False,
                            queue_num=qn[0] % 4)
                        qn[0] += 1

                def gather_b(s):
                    tb = int(T[s, 1])
                    ncb = int(NCH[s, 1])
                    sb = int(t_off[s, 1])
                    mb = mpoolB.tile([128, ncb, D], dt, tag="mb")
                    tiles[('b', s)] = mb
                    for g0 in range(0, tb, GS):
                        g1 = min(g0 + GS, tb)
                        nc.gpsimd.dma_gather(
                            mb[:, g0 // 128:g1 // 128, :], viewB,
                            ib_t[:, (sb + g0) // 16:(sb + g1) // 16],
                            g1 - g0, g1 - g0, D, single_packet=False,
                            queue_num=qn[0] % 4)
                        qn[0] += 1

                def compute(s):
                    b0, b1 = SGS[s]
                    nca = int(NCH[s, 0])
                    ncb = int(NCH[s, 1])
                    ma = tiles[('a', s)]
                    mb = tiles[('b', s)]
                    ma16 = m16pool.tile([128, nca, D], bf, tag="ma16")
                    mb16 = m16pool.tile([128, ncb, D], bf, tag="mb16")
                    nc.scalar.copy(out=ma16[:], in_=ma[:])
                    nc.scalar.copy(out=mb16[:], in_=mb[:])

                    # on-chip one-hot generation for this supergroup
                    na = int(nmm[s, 0])
                    nb_ = int(nmm[s, 1])
                    oa = int(mm_off[s, 0])
                    ob = int(mm_off[s, 1])
                    qa = qpool.tile([128, na, 128], bf, tag="qa")
                    qb = qpool.tile([128, nb_, 128], bf, tag="qb")
                    nc.vector.tensor_tensor(
                        out=qa[:],
                        in0=qv_t[:, oa:oa + na].unsqueeze(2)
                            .broadcast_to([128, na, 128]),
                        in1=iota_t[:].unsqueeze(1)
                            .broadcast_to([128, na, 128]),
                        op=mybir.AluOpType.is_equal)
                    nc.vector.tensor_tensor(
                        out=qb[:],
                        in0=qv_t[:, ob:ob + nb_].unsqueeze(2)
                            .broadcast_to([128, nb_, 128]),
                        in1=iota_t[:].unsqueeze(1)
                            .broadcast_to([128, nb_, 128]),
                        op=mybir.AluOpType.is_equal)

                    for b in range(b0, b1):
                        chain = chains[s][b]
                        ps = ppool.tile([128, D], dt, tag="ps")
                        tot = len(chain)
                        for j, (h, k, col) in enumerate(chain):
                            qt = qa if h == 0 else qb
                            mt = ma16 if h == 0 else mb16
                            nc.tensor.matmul(
                                ps[:], qt[:, col, :], mt[:, k, :],
                                start=(j == 0), stop=(j == tot - 1))
                        # u_new = w * (agg + u) + y0
                        nc.vector.tensor_tensor(
                            out=u_nxt[:, b, :], in0=ps[:], in1=u_cur[:, b, :],
                            op=mybir.AluOpType.add)
                        # (two tensor_tensor ops: the tensor_scalar family can
                        # enter 2-port DVE perf mode, which locks GpSimd out
                        # of the shared SBUF port mid-gather-gen)
                        nc.vector.tensor_tensor(
                            out=u_nxt[:, b, :], in0=u_nxt[:, b, :],
                            in1=wt[:, b:b + 1].broadcast_to([128, D]),
                            op=mybir.AluOpType.mult)
                        nc.vector.tensor_tensor(
                            out=u_nxt[:, b, :], in0=u_nxt[:, b, :],
                            in1=y0t[:, b, :],
                            op=mybir.AluOpType.add)

                    if hop < K - 1:
                        if b1 <= NBA:
                            nc.sync.dma_start(
                                node_ap(bounceA)[:, b0:b1, :],
                                u_nxt[:, b0:b1, :])
                        else:
                            nc.sync.dma_start(
                                node_ap(bounceB)[:, b0 - NBA:b1 - NBA, :],
                                u_nxt[:, b0:b1, :])
                        if b1 == NBA:
                            # blocks 0-24 done: fire AG-A mid-hop
                            nc.gpsimd.collective_compute(
                                "AllGather", mybir.AluOpType.bypass,
                                replica_groups=[list(range(C))],
                                ins=[bounceA[:]],
                                outs=[urepsA[(hop + 1) % 2][:]],
                            )

                # Stagger B-stream gathers two supergroups behind A-stream:
                # B-gathers wait on AG-B of the previous hop, and the GpSimd
                # engine FIFO is in-order, so a blocked B-gather must not sit
                # in front of ready A-gathers.
                LAG = 2
                for s in range(NSG + LAG):
                    if s < NSG:
                        gather_a(s)
                    if s >= LAG:
                        gather_b(s - LAG)
                        compute(s - LAG)

                if hop < K - 1:
                    nc.gpsimd.collective_compute(
                        "AllGather", mybir.AluOpType.bypass,
                        replica_groups=[list(range(C))],
                        ins=[bounceB[:]], outs=[urepsB[(hop + 1) % 2][:]],
                    )
                u_cur, u_nxt = u_nxt, u_cur

            # epilogue: out = relu(u * sqrt(deg))
            ot = u_nxt  # reuse the dead double buffer
            nc.vector.tensor_tensor(
                out=ot[:], in0=u_cur[:],
                in1=sqt[:].unsqueeze(2).broadcast_to([128, NB, D]),
                op=mybir.AluOpType.mult)
            nc.vector.tensor_scalar_max(out=ot[:], in0=ot[:], scalar1=0.0)
            nc.sync.dma_start(node_ap(out_d), ot[:])

    nc.compile()
    return nc


def kernel(x, edge_index):
    (meta, idx_a_w, idx_b_w, qv16, iota16,
     u0_s, y0_s, w_s, sq_s) = _host_prep(x, edge_index)
    nc = _build_nc(meta)

    from concourse.bass_utils import run_bass_kernel_spmd

    in_maps = []
    for c in range(C):
        in_maps.append({
            "u0": u0_s[c], "y0": y0_s[c], "w": w_s[c], "sq": sq_s[c],
            "idx_a": idx_a_w[c], "idx_b": idx_b_w[c],
            "qv": qv16[c], "iota": iota16,
        })

    ntff_dir = os.environ.get("APPNP_NTFF_DIR")
    if ntff_dir:
        from trn_agent_boot.trn_boot import _ntff_profile_via_ctypes
        hook = _ntff_profile_via_ctypes('/opt/axon/libaxon_pjrt.so')
        os.makedirs(ntff_dir, exist_ok=True)
        with hook(ntff_dir, None):
            res = run_bass_kernel_spmd(nc, in_maps, core_ids=list(range(C)))
    else:
        res = run_bass_kernel_spmd(nc, in_maps, core_ids=list(range(C)))

    out = np.empty((N, D), dtype=np.float32)
    for c in range(C):
        out[c * NS:(c + 1) * NS] = res.results[c]["out"][:NS]
    return out
